# revision 40
# baseline (speedup 1.0000x reference)
"""Trainium2 Bass kernel for CharModel ragged segment-mean + pos embedding.

Computation (per sample):
  out[j, :] = mean(feats[start_j:end_j, :]) * valid_j + pos_table[pos_j]
where the ragged segments are given by sorted word start offsets.

Strategy (all fp32):
  - Host precomputes per-char metadata: word_id[c] (which word each char
    belongs to, -1 for padding chars) and wrec[c] = 1/len(word(c)).
  - Device builds a weighted one-hot matrix M'[c, j] = (word_id[c]==j)/len_j
    with one fused DVE tensor_scalar op per 128-char tile, then the PE
    computes mean[W, D] = M'.T @ feats directly in PSUM.  The pos embedding
    is added by accumulating onehot(pos).T @ pos_table into the same PSUM.
  - Data parallel over batch: 8 NeuronCores x 4 samples each, one shared
    SPMD program, per-core input maps.

Walrus ISA wait-slot limits dealt with throughout: matmul (S3_LW),
tensor_scalar (S3D3_TS) and DMA (PSEUDO_DMA_DIRECT2D) instructions can
carry only ONE semaphore wait each.  Hence:
  - all constants/metadata (iota row, word metadata, pos one-hot,
    pos_table) ship in ONE packed tensor loaded by one early SWDGE DMA;
  - a per-sample 1x1 "gate" matmul (forced first in PE order via
    add_dep_helper, writing a dedicated never-overlapping PSUM region)
    alone carries the DVE wait for the lhsT builds and, transitively, the
    previous sample's PSUM bank releases;
  - feats tiles and output staging tiles get enough pool bufs that no
    slot is ever reused (no WAR waits on DMAs/copies);
  - one output DMA per sample (5 SWDGE DMAs total over 8 queues -> no
    queue-FIFO reuse waits alongside the data wait).
"""

import sys

if "/opt/trn_rl_repo" not in sys.path:
    sys.path.insert(0, "/opt/trn_rl_repo")

import numpy as np

import bass_rust
import concourse.bass as bass
import concourse.mybir as mybir
from concourse.tile import TileContext
from concourse.tile_sem_assignment import N_PROCS


class ChunkedDrainTileContext(TileContext):
    """TileContext whose kernel-tail drain is split into several drain
    instructions with one sem wait each (the CTRL_NO ISA struct rejects
    multi-wait drains here)."""

    DRAIN_CHUNK = 1

    def _drain_and_barrier(self, tick_clock, wait_clock):
        gc = tick_clock.global_clock
        ticks = [gc.peek_next(i) - 1 for i in range(N_PROCS)]
        active = [i for i, t in enumerate(ticks) if t > 0]
        for i in range(0, len(active), self.DRAIN_CHUNK):
            chunk = set(active[i : i + self.DRAIN_CHUNK])
            part = [ticks[j] if j in chunk else 0 for j in range(N_PROCS)]
            d = self.nc.sync.drain()
            wait_clock.add_sem_waits(
                d.ins, bass_rust.ScopedClock({None: bass_rust.VectorClock(part)})
            )
        self.nc.all_engine_barrier()
        assert self.sems is not None
        popped = self.nc._tile_sem_poison_stack.pop()
        assert popped is self._sem_poison
        self.nc.clear_and_free_semaphores(list(self.sems.allocated().values()))
        self.nc.all_engine_barrier()

B, S, D, W, NPOS = 32, 1024, 512 + 256, 512, 32  # D=768
N_CORES = 8
SPC = B // N_CORES  # samples per core
NT = S // 128  # char tiles per sample
NG = W // 128  # word groups per sample
CHUNKS = ((0, 384), (384, 384))  # D split for PSUM bank limit
F32 = mybir.dt.float32

BF16 = mybir.dt.bfloat16

# constpack layout (one [128, CPK_W] f32 tensor -> one DMA -> single-wait deps)
CPK_IOTA = 0  # [128, W]: iota row 0..W-1 replicated across partitions
CPK_META = W  # [128, NT*SPC]: per sample s: word-id per char, tile cols
CPK_WREC = W + NT * SPC  # [128, NG*SPC]: per sample s: 1/len per word
CPK_W = CPK_WREC + NG * SPC

# bf16 constpack: rows 32s:32s+32 per sample s
CPB_POHH = 0  # [*, W]: onehot(pos_j) * bf16_hi(len_j)
CPB_POHL = W  # [*, W]: onehot(pos_j) * bf16_lo(len_j)
CPB_TABH = 2 * W  # [*, D]: bf16_hi(pos_table)
CPB_TABL = 2 * W + D  # [*, D]: bf16_lo(pos_table)
CPB_W = 2 * W + 2 * D


def _build_program(sched):
    """sched[s][g] = tuple of char-tile indices whose chars can touch word
    group g of slot-s samples on ANY core (union schedule; the one-hot
    lhsT zeroes contributions from tiles/words not actually present on a
    given core).  Matmuls for (g, t) pairs outside the schedule multiply
    all-zero one-hot slices and are skipped entirely."""
    nc = bass.Bass()
    # feats as hi/lo bf16 pair, interleaved per char row: [..., 0:D] = hi,
    # [..., D:2D] = lo  (hi + lo reproduces fp32 to ~2^-16).
    feats = nc.declare_dram_parameter("feats", [SPC, S, 2 * D], BF16, False)
    constpack = nc.declare_dram_parameter("constpack", [128, CPK_W], F32, False)
    constpkb = nc.declare_dram_parameter("constpkb", [128, CPB_W], BF16, False)
    out = nc.declare_dram_parameter("out", [SPC, W, D], F32, True)

    dep = lambda a, b, why: bass_rust.add_dep_helper(
        a.ins, b.ins, sync=False, reason=why
    )

    n_lh = sum(
        len({t for g in range(NG) for t in sched[s][g]}) for s in range(SPC)
    )
    # Coalesce each sample's used char tiles into contiguous runs (max 4
    # tiles) -> one 3D-AP DMA per run: ~8 big DMAs instead of ~32, one per
    # HWDGE queue, so the SP sequencer's ~1us per-DMA issue cost stops
    # dominating the kernel head.
    MAXRUN = 4
    all_runs = {}
    from collections import Counter

    runcnt = Counter()
    for s in range(SPC):
        uts = sorted({t for g in range(NG) for t in sched[s][g]})
        runs = []
        i = 0
        while i < len(uts):
            j = i
            while (
                j + 1 < len(uts)
                and uts[j + 1] == uts[j] + 1
                and (j + 1 - i) < MAXRUN
            ):
                j += 1
            runs.append((uts[i], j - i + 1))
            i = j + 1
        all_runs[s] = runs
        for (_, L) in runs:
            runcnt[L] += 1
    with ChunkedDrainTileContext(nc) as tc:
        with (
            tc.tile_pool(name="const", bufs=1) as cpool,
            tc.tile_pool(name="feat", bufs=SPC * NT) as fpool,
            tc.tile_pool(name="lhs", bufs=n_lh) as lpool,
            tc.tile_pool(name="outsb", bufs=SPC) as opool,
            tc.tile_pool(name="psum", bufs=2 * NG - 2, space="PSUM") as ppool,
            tc.tile_pool(name="gatep", bufs=1, space="PSUM") as gpool,
            tc.tile_pool(name="warmp", bufs=1, space="PSUM") as wpool,
        ):
            cpk = cpool.tile([128, CPK_W], F32)
            nc.gpsimd.dma_start(out=cpk[:, :], in_=constpack[:, :])
            cpb = cpool.tile([128, CPB_W], BF16)
            nc.gpsimd.dma_start(out=cpb[:, :], in_=constpkb[:, :])
            iota_f = cpk[:, CPK_IOTA : CPK_IOTA + W]
            # ACT probe: observe the constpack DMA tick on the Scalar engine
            # so the per-unit ACT output copies carry only their PE wait.
            act_probe = cpool.tile([1, 1], F32)
            nc.scalar.copy(act_probe[0:1, 0:1], cpk[0:1, 0:1])
            pl_probe = cpool.tile([1, SPC], F32)
            # PE warm-up: ~6us of fat fp32 matmuls (dependent only on the
            # constpack DMA) run during the DMA ramp and trip the HAM clock
            # gate to K=8/8 before the real matmuls start.  Without this the
            # PE sometimes stays at 1.2GHz for the whole kernel.
            wps = wpool.tile([1, 512], F32)
            for wi in range(8):
                nc.tensor.matmul(
                    wps[0:1, :],
                    cpk[:, 0:1],
                    cpk[:, 0:512],
                    start=(wi == 0),
                    stop=(wi == 7),
                    skip_group_check=True,
                )
            # One persistent PSUM bank for the gates; each gate writes a
            # disjoint region so gates never carry a WAW drain wait.
            gate_t = gpool.tile([128, 4 * SPC], F32)

            prev_ob = None  # previous sample's output staging buffer
            for s in range(SPC):
                last_dve_copy = None
                last_act_copy = None
                used_tiles = sorted({t for g in range(NG) for t in sched[s][g]})
                fts, lhs = {}, {}
                first_build = True
                for (t0, L) in all_runs[s]:
                    ftr = fpool.tile(
                        [128, L, 2 * D],
                        BF16,
                        tag=f"ftr{L}",
                        bufs=runcnt[L],
                        name=f"ftr_{s}_{t0}",
                    )
                    nc.sync.dma_start(
                        out=ftr[:, :, :],
                        in_=feats[s, 128 * t0 : 128 * (t0 + L), :].rearrange(
                            "(i p) d -> p i d", p=128
                        ),
                    )
                    for i in range(L):
                        fts[t0 + i] = ftr[:, i, :]
                for t in used_tiles:
                    lh = lpool.tile([128, W], BF16, tag="lh", name=f"lh_{s}_{t}")
                    wcol = CPK_META + NT * s
                    bi = nc.vector.tensor_scalar(
                        lh[:, :],
                        iota_f,
                        cpk[:, wcol + t : wcol + t + 1],
                        None,
                        op0=mybir.AluOpType.is_equal,
                    )
                    lhs[t] = lh

                # Gate A: 1x1x1 matmul reading the last lhsT build; forced
                # first in PE order so it alone carries the DVE wait for
                # this sample's builds.
                last_lh = lhs[used_tiles[-1]]
                gate = nc.tensor.matmul(
                    gate_t[0:1, s : s + 1],
                    last_lh[0:1, 0:1],
                    last_lh[0:1, 0:1],
                    start=True,
                    stop=True,
                    skip_group_check=True,
                )
                # Gate B: reads the previous sample's output staging buffer
                # (written by its LAST PSUM->SBUF op, which a dep chain keeps
                # last on DVE), so this one wait covers all of the previous
                # sample's PSUM bank releases.  The builds above no longer
                # depend on those releases, so they run early and the PE does
                # not stall at sample boundaries.
                if prev_ob is not None:
                    # last DVE copy writes unit 6 -> ob[:, 3D : 3D+384];
                    # last ACT copy writes unit 7 -> ob[:, 3D+384 : 4D]
                    bgate = nc.tensor.matmul(
                        gate_t[0:1, SPC + s : SPC + s + 1],
                        prev_ob[0:1, 3 * D : 3 * D + 1],
                        prev_ob[0:1, 3 * D : 3 * D + 1],
                        start=True,
                        stop=True,
                        skip_group_check=True,
                    )
                    bgate2 = nc.tensor.matmul(
                        gate_t[0:1, 2 * SPC + s : 2 * SPC + s + 1],
                        prev_ob[0:1, NG * D - 1 : NG * D],
                        prev_ob[0:1, NG * D - 1 : NG * D],
                        start=True,
                        stop=True,
                        skip_group_check=True,
                    )
                else:
                    bgate = bgate2 = None

                ob = opool.tile([128, NG * D], F32, tag="ob", name=f"ob_{s}")
                for g in range(NG):
                    tiles_g = sched[s][g]
                    for ci, (c0, cn) in enumerate(CHUNKS):
                        ps = ppool.tile(
                            [128, cn], F32, tag="ps", name=f"ps_{s}_{g}_{ci}"
                        )
                        for k, t in enumerate(tiles_g):
                            for hl in range(2):  # feats hi then lo
                                mm = nc.tensor.matmul(
                                    ps[:, :],
                                    lhs[t][:, 128 * g : 128 * (g + 1)],
                                    fts[t][:, hl * D + c0 : hl * D + c0 + cn],
                                    start=(k == 0 and hl == 0),
                                    stop=False,
                                    skip_group_check=True,
                                )
                                dep(mm, gate, "matmuls after sample gate")
                                if bgate is not None:
                                    dep(mm, bgate, "matmuls after bank gate")
                                    dep(mm, bgate2, "matmuls after bank gate2")
                        # pos contribution scaled by len so the final 1/len
                        # multiply leaves exactly pos_table[pos]:
                        #   (lenhi@tabhi + lenhi@tablo + lenlo@tabhi)
                        # (the lenlo@tablo cross term is ~2^-16 relative).
                        r0, r1 = 32 * s, 32 * s + 32
                        pos_terms = (
                            (CPB_POHH, CPB_TABH),
                            (CPB_POHH, CPB_TABL),
                            (CPB_POHL, CPB_TABH),
                        )
                        for pi, (lcol, rcol) in enumerate(pos_terms):
                            mm = nc.tensor.matmul(
                                ps[:, :],
                                cpb[r0:r1, lcol + 128 * g : lcol + 128 * (g + 1)],
                                cpb[r0:r1, rcol + c0 : rcol + c0 + cn],
                                start=(len(tiles_g) == 0 and pi == 0),
                                stop=(pi == 2),
                                skip_group_check=True,
                                tile_position=(32 * s, 0),
                            )
                            dep(mm, gate, "pos matmul after sample gate")
                            if bgate is not None:
                                dep(mm, bgate, "pos matmul after bank gate")
                                dep(mm, bgate2, "pos matmul after bank gate2")
                        unit = 2 * g + ci
                        recip_ap = cpk[
                            :, CPK_WREC + NG * s + g : CPK_WREC + NG * s + g + 1
                        ]
                        if unit % 2 == 0:
                            cp = nc.vector.tensor_scalar(
                                ob[:, g * D + c0 : g * D + c0 + cn],
                                ps[:, :],
                                recip_ap,
                                None,
                                op0=mybir.AluOpType.mult,
                            )
                            if last_dve_copy is not None:
                                dep(cp, last_dve_copy, "DVE copy order")
                            last_dve_copy = cp
                        else:
                            cp = nc.scalar.activation(
                                ob[:, g * D + c0 : g * D + c0 + cn],
                                ps[:, :],
                                mybir.ActivationFunctionType.Copy,
                                scale=recip_ap,
                            )
                            if last_act_copy is not None:
                                dep(cp, last_act_copy, "ACT copy order")
                            last_act_copy = cp
                # Pool probe: observe the last DVE copy's tick on the Pool
                # engine so the output DMA carries only the ACT copy wait.
                nc.gpsimd.tensor_copy(
                    pl_probe[0:1, s : s + 1], ob[0:1, 3 * D : 3 * D + 1]
                )
                nc.gpsimd.dma_start(
                    out=out[s].rearrange("(g p) d -> p g d", p=128),
                    in_=ob[:, :].rearrange("p (g d) -> p g d", g=NG),
                )
                prev_ob = ob
    return nc


_PROGRAM_CACHE = {}


def _get_program(sched):
    key = tuple(tuple(tuple(g) for g in s) for s in sched)
    if key not in _PROGRAM_CACHE:
        _PROGRAM_CACHE[key] = _build_program(sched)
    return _PROGRAM_CACHE[key]


def _assign_slots(spans):
    """Assign the B samples to (slot, core) so that the per-slot UNION of
    (group, char-tile) matmul footprints is small: sort by profile, then
    cheap local-search swaps."""
    import random

    def union_cost(assign):
        total = 0
        for slot in assign:
            u = np.zeros((NG, NT), bool)
            for i in slot:
                for (g, t0, t1) in spans[i][0]:
                    u[g, t0 : t1 + 1] = True
            total += int(u.sum())
        return total

    order = sorted(range(B), key=lambda i: spans[i][1])
    assign = [[order[s * N_CORES + c] for c in range(N_CORES)] for s in range(SPC)]
    rng = random.Random(0)
    best_cost = union_cost(assign)
    for _ in range(3000):
        s1, s2 = rng.randrange(SPC), rng.randrange(SPC)
        if s1 == s2:
            continue
        i1, i2 = rng.randrange(N_CORES), rng.randrange(N_CORES)
        assign[s1][i1], assign[s2][i2] = assign[s2][i2], assign[s1][i1]
        c = union_cost(assign)
        if c <= best_cost:
            best_cost = c
        else:
            assign[s1][i1], assign[s2][i2] = assign[s2][i2], assign[s1][i1]
    return assign


def _prep_inputs(feats, word_lens, seq_len, pos, pos_table):
    """Host-side metadata prep + batch sharding -> per-core input maps,
    union matmul schedule, and the sample->(slot, core) assignment."""
    feats = np.ascontiguousarray(np.asarray(feats), dtype=np.float32)
    word_lens = np.asarray(word_lens).astype(np.int64)
    seq_len = np.asarray(seq_len).astype(np.int64)
    pos = np.asarray(pos).astype(np.int64)
    pos_table = np.ascontiguousarray(np.asarray(pos_table), dtype=np.float32)

    import ml_dtypes

    bf16 = ml_dtypes.bfloat16
    wid = np.full((B, S), -1.0, np.float32)
    wrecw = np.zeros((B, W), np.float32)  # 1/len per word (0 for padding)
    lenw = np.zeros((B, W), np.float32)  # len per word (0 for padding)
    posoh = np.zeros((B, NPOS, W), np.float32)
    spans = []  # per sample: ([(g, t0, t1), ...], profile_key)
    for i in range(B):
        wl = word_lens[i]
        sl = int(seq_len[i])
        valid = wl != 0
        valid[0] = True
        ridx = np.nonzero(valid)[0]  # real words (contiguous prefix by construction)
        starts = wl[ridx]
        n = len(ridx)
        nxt = np.append(starts[1:], 0)
        ends = np.where(nxt == 0, sl, nxt)
        lens = np.maximum(ends - starts, 1)
        cidx = np.arange(sl)
        cwid = np.searchsorted(starts, cidx, side="right") - 1
        wid[i, :sl] = ridx[cwid].astype(np.float32)
        wrecw[i, ridx] = 1.0 / lens.astype(np.float32)
        lenw[i, ridx] = lens.astype(np.float32)
        posoh[i, pos[i], np.arange(W)] = 1.0  # one-hot part
        sp = []
        for g in range(NG):
            w0 = 128 * g
            if w0 >= n:
                continue
            w1 = min(128 * (g + 1), n)
            c0, c1 = int(starts[w0]), int(ends[w1 - 1])
            sp.append((g, c0 // 128, (c1 - 1) // 128))
        spans.append((sp, (n, sl)))

    assign = _assign_slots(spans)
    sched = []
    for s in range(SPC):
        u = np.zeros((NG, NT), bool)
        for i in assign[s]:
            for (g, t0, t1) in spans[i][0]:
                u[g, t0 : t1 + 1] = True
        sched.append(tuple(tuple(np.nonzero(u[g])[0].tolist()) for g in range(NG)))
    sched = tuple(sched)

    # [B, S] -> [B, 128, NT]: per-partition scalar columns per char tile
    widT = wid.reshape(B, NT, 128).transpose(0, 2, 1)
    # 1/len per word -> [B, 128, NG] per-partition scalars per word group
    wrecwT = wrecw.reshape(B, NG, 128).transpose(0, 2, 1)
    iota_row = np.broadcast_to(np.arange(W, dtype=np.float32), (128, W))

    feats_hi = feats.astype(bf16)
    feats_lo = (feats - feats_hi.astype(np.float32)).astype(bf16)
    len_hi = lenw.astype(bf16)
    len_lo = (lenw - len_hi.astype(np.float32)).astype(bf16)
    tab_hi = pos_table.astype(bf16)
    tab_lo = (pos_table - tab_hi.astype(np.float32)).astype(bf16)

    in_maps = []
    for c in range(N_CORES):
        cpk = np.zeros((128, CPK_W), np.float32)
        cpk[:, CPK_IOTA : CPK_IOTA + W] = iota_row
        cpb = np.zeros((128, CPB_W), bf16)
        feats_c = np.empty((SPC, S, 2 * D), bf16)
        for s in range(SPC):
            i = assign[s][c]
            feats_c[s, :, :D] = feats_hi[i]
            feats_c[s, :, D:] = feats_lo[i]
            cpk[:, CPK_META + NT * s : CPK_META + NT * (s + 1)] = widT[i]
            cpk[:, CPK_WREC + NG * s : CPK_WREC + NG * (s + 1)] = wrecwT[i]
            r0, r1 = 32 * s, 32 * s + 32
            cpb[r0:r1, CPB_POHH : CPB_POHH + W] = posoh[i] * len_hi[i][None, :]
            cpb[r0:r1, CPB_POHL : CPB_POHL + W] = posoh[i] * len_lo[i][None, :]
            cpb[r0:r1, CPB_TABH : CPB_TABH + D] = tab_hi
            cpb[r0:r1, CPB_TABL : CPB_TABL + D] = tab_lo
        in_maps.append({"feats": feats_c, "constpack": cpk, "constpkb": cpb})
    return in_maps, sched, assign


def _run(in_maps, sched, assign, trace=False):
    from concourse.bass_utils import run_bass_kernel_spmd

    nc = _get_program(sched)
    res = run_bass_kernel_spmd(nc, in_maps, list(range(N_CORES)), trace=trace)
    out = np.empty((B, W, D), np.float32)
    for c in range(N_CORES):
        for s in range(SPC):
            out[assign[s][c]] = res.results[c]["out"][s]
    return out, res


def kernel(feats, word_lens, seq_len, pos, pos_table):
    in_maps, sched, assign = _prep_inputs(feats, word_lens, seq_len, pos, pos_table)
    out, _ = _run(in_maps, sched, assign, trace=False)
    return out
